# revision 8
# baseline (speedup 1.0000x reference)
"""Trainium2 Bass kernel for nn_Attention_59691455480358 (sparse CLS attention).

Math: reference computes softmax over logits[b, n] = (x[b,0]@W_q) . (x[b,1+n]@W_k) * C^-0.5
for n in [0, 2048).  Only the CLS query row matters and V is unused, so fold the
K-projection into the query side:

    t[b]        = W_k @ (x[b,0,:] @ W_q)          # [C] per example
    logits[b,n] = x[b,1+n,:] . t[b]               # row dot-products
    out[b]      = softmax(logits[b] * C^-0.5)

Sharding: pure data parallel — batch 16 over 8 NeuronCores (2 examples/core),
w_qkv replicated.  Per core the kernel is DMA-bound (~17 MB of x) with the row
dot-products on DVE via fused tensor_tensor_reduce, softmax via ACT exp+accum
and a PE all-ones matmul for the cross-partition sum broadcast.

Row->partition mapping: rows 1..2048 of x[b] are viewed as [128, 16*1024]
(partition p holds rows 16p+1 .. 16p+16), so logit tile L[p, f] is the logit for
n = 16p + f and the output DMA writes 64B-contiguous runs per partition.
"""
import os
import sys

for _p in ("/opt/trn_rl_repo", "/root/.axon_site", "/root/.axon_site/_ro/trn_rl_repo",
           "/root/.axon_site/_ro/pypackages"):
    if _p not in sys.path:
        sys.path.append(_p)

from contextlib import ExitStack

import numpy as np

import concourse.bass as bass  # noqa: F401  (registers rust bindings)
import concourse.tile as tile
from concourse import bacc, mybir
from concourse import bass_utils
from concourse.bass_interp import get_hw_module
from concourse.masks import make_identity

N_CORES = 8
B, N, C = 16, 2049, 1024
B_LOC = B // N_CORES        # 2 examples per core
P = 128                     # SBUF partitions
CT = C // P                 # 8 c-tiles
NT = 16                     # free-dim logits per partition (128*16 = 2048 rows)
G = 4                       # rows per x DMA group -> [128, 4096] tiles
NG = NT // G                # 4 groups per example
F32 = mybir.dt.float32
F32R = mybir.dt.float32r
USE_F32R = True             # fp32r matmul: 4x faster on PE, slightly reduced precision


def _mm_dt(ap):
    return ap.bitcast(F32R) if USE_F32R else ap


def build_nc():
    """Build + compile the per-core Bass program (identical on all 8 cores)."""
    nc = bacc.Bacc("TRN2", target_bir_lowering=False, debug=False,
                   enable_asserts=True, num_devices=N_CORES)

    x_d = nc.dram_tensor("x", [B_LOC, N, C], F32, kind="ExternalInput").ap()
    w_d = nc.dram_tensor("w", [C, 3 * C], F32, kind="ExternalInput").ap()
    o_d = nc.dram_tensor("o", [B_LOC, N - 1], F32, kind="ExternalOutput").ap()

    with tile.TileContext(nc) as tc, ExitStack() as ctx:
        sing = ctx.enter_context(tc.tile_pool(name="sing", bufs=1))
        xp = ctx.enter_context(tc.tile_pool(name="xp", bufs=4))
        scr = ctx.enter_context(tc.tile_pool(name="scr", bufs=2))
        pst = ctx.enter_context(tc.tile_pool(name="pst", bufs=2, space="PSUM"))
        pss = ctx.enter_context(tc.tile_pool(name="pss", bufs=2, space="PSUM"))
        psb = ctx.enter_context(tc.tile_pool(name="psb", bufs=2, space="PSUM"))

        # --- constants -----------------------------------------------------
        ident = sing.tile([P, P], F32, tag="ident")
        make_identity(nc, ident[:])
        ones1_f = sing.tile([1, P], F32, tag="ones1_f")
        nc.gpsimd.memset(ones1_f[:], 1.0)
        ones1 = sing.tile([1, P], F32R, tag="ones1")        # K=1 broadcast lhsT
        nc.scalar.copy(ones1[:], ones1_f[:])
        ones128 = sing.tile([P, P], F32, tag="ones128")    # partition-sum lhsT
        nc.gpsimd.memset(ones128[:], 1.0)

        # --- weights: W_q|W_k natural layout, 8 x [128, 2048] -------------
        wqk = []
        for j in range(CT):
            t_ = sing.tile([P, 2 * C], F32, tag=f"wqk{j}")
            nc.sync.dma_start(t_[:], w_d[P * j:P * (j + 1), 0:2 * C])
            wqk.append(t_)

        # --- CLS rows x0 [2, 1024] and their transpose x0T [128, 2*8] ------
        x0 = sing.tile([B_LOC, C], F32, tag="x0")
        nc.sync.dma_start(x0[:], x_d[:, 0, :])
        x0T = sing.tile([P, B_LOC * CT], F32, tag="x0T")
        for j in range(CT):
            ps = pst.tile([P, B_LOC], F32, tag="tp")
            nc.tensor.transpose(ps[:], x0[:, P * j:P * (j + 1)], ident[:B_LOC, :B_LOC])
            nc.scalar.copy(x0T[:, B_LOC * j:B_LOC * (j + 1)], ps[:])

        # --- q_cls = x0 @ W_q  -> [2, 1024] --------------------------------
        q_sb = sing.tile([B_LOC, C], F32, tag="q_sb")
        for h in range(2):
            psq = pss.tile([B_LOC, 512], F32, tag="pssm")
            for j in range(CT):
                nc.tensor.matmul(psq[:],
                                 x0T[:, B_LOC * j:B_LOC * (j + 1)],
                                 wqk[j][:, 512 * h:512 * (h + 1)],
                                 start=(j == 0), stop=(j == CT - 1))
            nc.scalar.copy(q_sb[:, 512 * h:512 * (h + 1)], psq[:])

        # --- q_cls^T [128, 2*8] --------------------------------------------
        qT = sing.tile([P, B_LOC * CT], F32R, tag="qT")
        for m in range(CT):
            ps = pst.tile([P, B_LOC], F32, tag="tp")
            nc.tensor.transpose(ps[:], q_sb[:, P * m:P * (m + 1)], ident[:B_LOC, :B_LOC])
            nc.scalar.copy(qT[:, B_LOC * m:B_LOC * (m + 1)], ps[:])

        # --- W_k^T via PE transposes: wkt[m] = [128 (c_out chunk m), 1024 (c_in)]
        wkt = [sing.tile([P, C], F32R, tag=f"wkt{m}", name=f"wkt{m}")
               for m in range(CT)]
        for j in range(CT):          # c_in tile of W_k natural
            for m in range(CT):      # c_out chunk
                ps = pst.tile([P, P], F32, tag="tp")
                nc.tensor.transpose(
                    ps[:], wqk[j][:, C + P * m:C + P * (m + 1)], ident[:])
                nc.scalar.copy(wkt[m][:, P * j:P * (j + 1)], ps[:])

        # --- t[b] = W_k @ q_cls[b]  -> per-example [1, 1024] tiles ---------
        # (separate tiles so each has base partition 0 — PE operand rule)
        t_sb = [sing.tile([1, C], F32R, tag=f"t_sb{b}", name=f"t_sb{b}")
                for b in range(B_LOC)]
        for b in range(B_LOC):
            for h in range(2):
                ps_t = pss.tile([1, 512], F32, tag="pssm")
                for m in range(CT):
                    nc.tensor.matmul(ps_t[:],
                                     qT[:, B_LOC * m + b:B_LOC * m + b + 1],
                                     wkt[m][:, 512 * h:512 * (h + 1)],
                                     start=(m == 0), stop=(m == CT - 1))
                nc.scalar.copy(t_sb[b][:, 512 * h:512 * (h + 1)], ps_t[:])

        # --- broadcast t across partitions: tb[b] [128, 1024] --------------
        tb = []
        for b in range(B_LOC):
            tb_b = sing.tile([P, C], F32, tag=f"tb{b}", name=f"tb{b}")
            for h in range(2):
                ps_b = psb.tile([P, 512], F32, tag="psb")
                nc.tensor.matmul(ps_b[:],
                                 ones1[:],
                                 t_sb[b][:, 512 * h:512 * (h + 1)],
                                 start=True, stop=True)
                nc.scalar.copy(tb_b[:, 512 * h:512 * (h + 1)], ps_b[:])
            tb.append(tb_b)

        STAGE = int(os.environ.get("KSTAGE", "3"))
        if STAGE == 1:
            # t-chain only: dump tb values
            for b in range(B_LOC):
                nc.sync.dma_start(o_d[b].rearrange("(p f) -> p f", f=NT),
                                  tb[b][:, 0:NT])
        # --- big pass: logits + softmax per example ------------------------
        for b in range(B_LOC if STAGE >= 2 else 0):
            # rows 1..2048 viewed as [128 partitions, 16 rows * 1024]
            xb = x_d[b, 1:N, :].rearrange("(p f) c -> p (f c)", f=NT)
            Lb = sing.tile([P, NT], F32, tag=f"L{b}")
            for g in range(NG):
                xt = xp.tile([P, G * C], F32, tag="xg")
                nc.sync.dma_start(xt[:], xb[:, G * C * g:G * C * (g + 1)])
                for l in range(G):
                    f = G * g + l
                    s = scr.tile([P, C], F32, tag="scr")
                    nc.vector.scalar_tensor_tensor(
                        out=s[:],
                        in0=xt[:, C * l:C * (l + 1)],
                        scalar=1.0,
                        in1=tb[b][:],
                        op0=mybir.AluOpType.mult,
                        op1=mybir.AluOpType.mult,
                        accum_out=Lb[:, f:f + 1])

            if STAGE == 2:
                nc.sync.dma_start(o_d[b].rearrange("(p f) -> p f", f=NT), Lb[:])
                continue
            # softmax over all 2048 logits of example b (no max subtraction:
            # scaled logits are ~N(0,1); exp cannot overflow fp32)
            E = sing.tile([P, NT], F32, tag=f"E{b}")
            S = sing.tile([P, 1], F32, tag=f"S{b}")
            nc.scalar.activation(E[:], Lb[:], mybir.ActivationFunctionType.Exp,
                                 bias=0.0, scale=float(C ** -0.5), accum_out=S[:])
            psS = pss.tile([P, 1], F32, tag="pssm")
            nc.tensor.matmul(psS[:], ones128[:], S[:], start=True, stop=True)
            Rv = sing.tile([P, 1], F32, tag=f"R{b}")
            nc.vector.reciprocal(Rv[:], psS[:])
            Pb = sing.tile([P, NT], F32, tag=f"P{b}")
            nc.vector.tensor_scalar_mul(Pb[:], E[:], Rv[:])
            nc.sync.dma_start(o_d[b].rearrange("(p f) -> p f", f=NT), Pb[:])

    nc.compile()
    nc.m = get_hw_module(nc.m)
    return nc


_NC_CACHE = {}


def _get_nc():
    if "nc" not in _NC_CACHE:
        _NC_CACHE["nc"] = build_nc()
    return _NC_CACHE["nc"]


def _run(x, w_qkv, **kwargs):
    x = np.ascontiguousarray(np.asarray(x, dtype=np.float32))
    w = np.ascontiguousarray(np.asarray(w_qkv, dtype=np.float32))
    assert x.shape == (B, N, C) and w.shape == (C, 3 * C)
    nc = _get_nc()
    in_maps = [{"x": x[c * B_LOC:(c + 1) * B_LOC], "w": w} for c in range(N_CORES)]
    res = bass_utils.run_bass_kernel_spmd(nc, in_maps,
                                          core_ids=list(range(N_CORES)), **kwargs)
    out = np.concatenate([res.results[c]["o"] for c in range(N_CORES)], axis=0)
    return out, res


def kernel(x, w_qkv):
    out, _ = _run(x, w_qkv)
    return out


# revision 9
# speedup vs baseline: 1.1668x; 1.1668x over previous
"""Trainium2 Bass kernel for nn_Attention_59691455480358 (sparse CLS attention).

Math: the reference computes softmax over
    logits[b, n] = (x[b,0]@W_q) . (x[b,1+n]@W_k) * C^-0.5,  n in [0, 2048).
Only the CLS query row matters and V is unused, so fold the K-projection into
the query side:

    t[b]        = W_k @ (x[b,0,:] @ W_q)          # [C] per example
    logits[b,n] = x[b,1+n,:] . t[b]               # row dot-products
    out[b]      = softmax(logits[b] * C^-0.5)

Sharding: data parallel over batch (2 examples/core) for the heavy x pass; the
tiny t computation is sharded over the weight's c_out dim — each core loads
only a 128-column chunk of W_q/W_k (1 MB instead of 8 MB), computes a partial
t for ALL 16 examples, and a ReduceScatter(add) hands every core exactly its
own two t rows.  t is then partition-broadcast from DRAM and the row
dot-products run on DVE via fused scalar_tensor_tensor (out=x*t, accum=sum).
Softmax: ACT exp with fused accumulation, all-ones PE matmul for the
cross-partition sum (every partition receives the total), DVE reciprocal +
tensor_scalar multiply.

Row->partition mapping: rows 1..2048 of x[b] are viewed as [128, 16*1024]
(partition p holds rows 16p+1 .. 16p+16), so logit tile L[p, f] is the logit
for n = 16p + f and the output DMA writes 64B-contiguous runs per partition.
No max-subtraction in softmax: scaled logits are ~N(0,1) (weights are
1/sqrt(C)-scaled gaussians), exp cannot overflow fp32.
"""
import sys

for _p in ("/opt/trn_rl_repo", "/root/.axon_site", "/root/.axon_site/_ro/trn_rl_repo",
           "/root/.axon_site/_ro/pypackages"):
    if _p not in sys.path:
        sys.path.append(_p)

from contextlib import ExitStack

import numpy as np

import concourse.bass as bass  # noqa: F401
import concourse.tile as tile
from concourse import bacc, mybir
from concourse import bass_utils
from concourse.bass_interp import get_hw_module
from concourse.masks import make_identity

N_CORES = 8
B, N, C = 16, 2049, 1024
B_LOC = B // N_CORES        # 2 examples per core
P = 128                     # SBUF partitions
CT = C // P                 # 8 c tiles
NT = 16                     # logits per partition (128*16 = 2048 rows)
G = 4                       # rows per x DMA group -> [128, 4096] tiles
NG = NT // G
F32 = mybir.dt.float32


def build_nc():
    nc = bacc.Bacc("TRN2", target_bir_lowering=False, debug=False,
                   enable_asserts=True, num_devices=N_CORES)

    x_d = nc.dram_tensor("x", [B_LOC, N, C], F32, kind="ExternalInput").ap()
    x0a_d = nc.dram_tensor("x0all", [B, C], F32, kind="ExternalInput").ap()
    wq_d = nc.dram_tensor("wq", [C, P], F32, kind="ExternalInput").ap()
    wk_d = nc.dram_tensor("wk", [C, P], F32, kind="ExternalInput").ap()
    o_d = nc.dram_tensor("o", [B_LOC, N - 1], F32, kind="ExternalOutput").ap()

    with tile.TileContext(nc) as tc, ExitStack() as ctx:
        sing = ctx.enter_context(tc.tile_pool(name="sing", bufs=1))
        xp = ctx.enter_context(tc.tile_pool(name="xp", bufs=6))
        scr = ctx.enter_context(tc.tile_pool(name="scr", bufs=2))
        pst = ctx.enter_context(tc.tile_pool(name="pst", bufs=2, space="PSUM"))
        pss = ctx.enter_context(tc.tile_pool(name="pss", bufs=2, space="PSUM"))
        dram = ctx.enter_context(tc.tile_pool(name="dram", bufs=1, space="DRAM"))

        # --- weight chunk + CLS-row DMAs (small, land first) ---------------
        wq_sb = sing.tile([P, C], F32, tag="wq_sb")     # tile j at cols 128j..
        wk_sb = sing.tile([P, C], F32, tag="wk_sb")
        for j in range(CT):
            nc.sync.dma_start(wq_sb[:, P * j:P * (j + 1)], wq_d[P * j:P * (j + 1), :])
            nc.sync.dma_start(wk_sb[:, P * j:P * (j + 1)], wk_d[P * j:P * (j + 1), :])
        x0a = sing.tile([B, C], F32, tag="x0a")
        nc.sync.dma_start(x0a[:], x0a_d[:])

        ident = sing.tile([P, P], F32, tag="ident")
        make_identity(nc, ident[:])
        ones128 = sing.tile([P, P], F32, tag="ones128")
        nc.gpsimd.memset(ones128[:], 1.0)

        # --- x streaming DMAs (emitted early so they start at t=0) ---------
        xts = []
        for b in range(B_LOC):
            xb = x_d[b, 1:N, :].rearrange("(p f) c -> p (f c)", f=NT)
            for g in range(NG):
                xt = xp.tile([P, G * C], F32, tag="xg", name=f"xg{b}_{g}")
                nc.sync.dma_start(xt[:], xb[:, G * C * g:G * C * (g + 1)])
                xts.append(xt)

        # --- x0all^T: [128, 8*16] ------------------------------------------
        x0aT = sing.tile([P, CT * B], F32, tag="x0aT")
        for j in range(CT):
            ps = pst.tile([P, B], F32, tag="tp")
            nc.tensor.transpose(ps[:], x0a[:, P * j:P * (j + 1)], ident[:B, :B])
            nc.scalar.copy(x0aT[:, B * j:B * (j + 1)], ps[:])

        # --- q_cls^T chunk [128 (this core's c_out), 16] -------------------
        psq = pss.tile([P, B], F32, tag="psq")
        for j in range(CT):
            nc.tensor.matmul(psq[:], wq_sb[:, P * j:P * (j + 1)],
                             x0aT[:, B * j:B * (j + 1)],
                             start=(j == 0), stop=(j == CT - 1))
        qT = sing.tile([P, B], F32, tag="qT")
        nc.scalar.copy(qT[:], psq[:])

        # --- (W_k chunk)^T: [128 (c_out chunk), 1024 (c_in)] ---------------
        wkT = sing.tile([P, C], F32, tag="wkT")
        for j in range(CT):
            ps = pst.tile([P, P], F32, tag="tp")
            nc.tensor.transpose(ps[:], wk_sb[:, P * j:P * (j + 1)], ident[:])
            nc.scalar.copy(wkT[:, P * j:P * (j + 1)], ps[:])

        # --- partial t for all 16 examples: [16, 1024] ---------------------
        tp_sb = sing.tile([B, C], F32, tag="tp_sb")
        for h in range(2):
            ps_t = pss.tile([B, 512], F32, tag="ps_t")
            nc.tensor.matmul(ps_t[:], qT[:], wkT[:, 512 * h:512 * (h + 1)],
                             start=True, stop=True)
            nc.scalar.copy(tp_sb[:, 512 * h:512 * (h + 1)], ps_t[:])

        # --- ReduceScatter: every core receives its own 2 t rows -----------
        bounce_in = dram.tile([B, C], F32, tag="bounce_in")
        bounce_out = dram.tile([B_LOC, C], F32, tag="bounce_out")
        nc.sync.dma_start(bounce_in[:], tp_sb[:])
        nc.gpsimd.collective_compute(
            "ReduceScatter", mybir.AluOpType.add,
            replica_groups=[list(range(N_CORES))],
            ins=[bounce_in.opt()], outs=[bounce_out.opt()])

        # --- broadcast own t rows across partitions via DMA ----------------
        tb = []
        for b in range(B_LOC):
            tb_b = sing.tile([P, C], F32, tag=f"tb{b}", name=f"tb{b}")
            nc.sync.dma_start(tb_b[:],
                              bounce_out[b:b + 1, :].broadcast_to([P, C]))
            tb.append(tb_b)

        # --- big pass: fused row-dot products + softmax per example --------
        for b in range(B_LOC):
            Lb = sing.tile([P, NT], F32, tag=f"L{b}", name=f"L{b}")
            for g in range(NG):
                xt = xts[b * NG + g]
                for l in range(G):
                    f = G * g + l
                    s = scr.tile([P, C], F32, tag="scr")
                    nc.vector.scalar_tensor_tensor(
                        out=s[:], in0=xt[:, C * l:C * (l + 1)], scalar=1.0,
                        in1=tb[b][:],
                        op0=mybir.AluOpType.mult, op1=mybir.AluOpType.mult,
                        accum_out=Lb[:, f:f + 1])

            E = sing.tile([P, NT], F32, tag=f"E{b}", name=f"E{b}")
            S = sing.tile([P, 1], F32, tag=f"S{b}", name=f"S{b}")
            nc.scalar.activation(E[:], Lb[:], mybir.ActivationFunctionType.Exp,
                                 bias=0.0, scale=float(C ** -0.5), accum_out=S[:])
            psS = pss.tile([P, 1], F32, tag="psS")
            nc.tensor.matmul(psS[:], ones128[:], S[:], start=True, stop=True)
            Rv = sing.tile([P, 1], F32, tag=f"R{b}", name=f"R{b}")
            nc.vector.reciprocal(Rv[:], psS[:])
            Pb = sing.tile([P, NT], F32, tag=f"P{b}", name=f"P{b}")
            nc.vector.tensor_scalar_mul(Pb[:], E[:], Rv[:])
            nc.sync.dma_start(o_d[b].rearrange("(p f) -> p f", f=NT), Pb[:])

    nc.compile()
    nc.m = get_hw_module(nc.m)
    return nc


_NC_CACHE = {}


def _get_nc():
    if "nc" not in _NC_CACHE:
        _NC_CACHE["nc"] = build_nc()
    return _NC_CACHE["nc"]


def _run(x, w_qkv, **kwargs):
    x = np.ascontiguousarray(np.asarray(x, dtype=np.float32))
    w = np.asarray(w_qkv, dtype=np.float32)
    assert x.shape == (B, N, C) and w.shape == (C, 3 * C)
    x0all = np.ascontiguousarray(x[:, 0, :])
    nc = _get_nc()
    in_maps = []
    for c in range(N_CORES):
        in_maps.append({
            "x": x[c * B_LOC:(c + 1) * B_LOC],
            "x0all": x0all,
            "wq": np.ascontiguousarray(w[:, P * c:P * (c + 1)]),
            "wk": np.ascontiguousarray(w[:, C + P * c:C + P * (c + 1)]),
        })
    res = bass_utils.run_bass_kernel_spmd(nc, in_maps,
                                          core_ids=list(range(N_CORES)), **kwargs)
    out = np.concatenate([res.results[c]["o"] for c in range(N_CORES)], axis=0)
    return out, res


def kernel(x, w_qkv):
    out, _ = _run(x, w_qkv)
    return out
